# revision 1
# baseline (speedup 1.0000x reference)
"""Trainium2 Bass kernel for the 2-layer tanh RNN (nn_DeeperRNN).

Strategy
--------
The T=512 recurrence is inherently serial (batch=1), so the program is
replicated on all 8 NeuronCores (identical SPMD program + identical data;
result read from core 0).  The win comes from restructuring:

  phase A:  A1 = X @ W_i2h1.T + b_i2h1 + b_h2h1          (batched matmul)
  phase B:  h1_t = tanh(A1_t + W_h2h1 h1_{t-1})          (512 serial steps)
  phase C:  A2 = H1 @ W_i2h2.T + b_i2h2 + b_h2h2         (batched matmul)
  phase D:  h2_t = tanh(A2_t + W_h2h2 h2_{t-1})          (512 serial steps)
  phase E:  out = h2_T @ W_h2o2.T + b_h2o2

The per-step gemv streams the (bf16) recurrent weight matrix through the
PE as the moving operand with the tiny h vector as the stationary operand,
using 4 column-group tiles (tile_position) for 4 concurrent streams.
The gemv output lands free-major on psum rows {0,32,64,96}; a DVE 32x32
block transpose flips it back to partition-major h-slots, with the j-axis
of every weight matrix host-side permuted so the transpose lands exactly
on the natural slot layout.  tanh runs on ScalarE over the strided
transposed columns.  Biases and the per-step A-term are folded into the
PSUM accumulation as rank-1 matmuls (one-hot / ones stationaries).
"""

import sys
import numpy as np
import ml_dtypes

sys.path.insert(0, "/opt/trn_rl_repo")

import concourse.bass as bass  # noqa: E402
import concourse.mybir as mybir  # noqa: E402
import concourse.bacc as bacc  # noqa: E402
import concourse.tile as tile  # noqa: E402
import concourse.bass_utils as bass_utils  # noqa: E402
from contextlib import ExitStack  # noqa: E402

BF16 = mybir.dt.bfloat16
F32 = mybir.dt.float32
Tanh = mybir.ActivationFunctionType.Tanh

T, IN, H, OUT = 512, 1024, 2048, 1024
NCHUNK = H // 128  # 16


def _host_prep(inputs):
    bf = ml_dtypes.bfloat16
    f32 = np.float32

    def perm_out_axis(a):
        # permute last axis: col (g, J, a2) = g*512 + 32*J + a2 <- row 128J + 32g + a2
        s = a.shape[:-1]
        return np.ascontiguousarray(
            a.reshape(*s, 16, 4, 32).swapaxes(-3, -2).reshape(*s, 2048)
        )

    def prep_wh(w):  # W [j, i] -> [128p, (c*4+g)*512 + J*32 + a2]
        wt = np.asarray(w, f32).T
        return np.ascontiguousarray(
            wt.reshape(16, 128, 16, 4, 32)
            .transpose(1, 0, 3, 2, 4)
            .reshape(128, 16 * 4 * 512)
            .astype(bf)
        )

    def pm(a, part=128):  # [K, N] -> [128, (K//128)*N] chunked partition-major
        k, n = a.shape
        return np.ascontiguousarray(
            a.reshape(k // part, part, n).transpose(1, 0, 2).reshape(part, -1)
        )

    x = np.asarray(inputs["word"], f32).reshape(T, IN)
    return {
        "xt": pm(np.ascontiguousarray(x.T).astype(bf)),
        "w1t": pm(perm_out_axis(np.asarray(inputs["W_i2h1"], f32).T).astype(bf)),
        "wi2t": pm(perm_out_axis(np.asarray(inputs["W_i2h2"], f32).T).astype(bf)),
        "wh1": prep_wh(inputs["W_h2h1"]),
        "wh2": prep_wh(inputs["W_h2h2"]),
        "wo2t": pm(np.asarray(inputs["W_h2o2"], f32).T.astype(bf)),
        "b1": perm_out_axis(
            np.asarray(inputs["b_i2h1"], f32) + np.asarray(inputs["b_h2h1"], f32)
        ).reshape(1, H).astype(bf),
        "b2": perm_out_axis(
            np.asarray(inputs["b_i2h2"], f32) + np.asarray(inputs["b_h2h2"], f32)
        ).reshape(1, H).astype(bf),
        "bo": np.asarray(inputs["b_h2o2"], f32).reshape(1, OUT).astype(bf),
        "ident": np.eye(128, dtype=bf),
        "ones_row": np.ones((1, 128), dtype=bf),
    }


_INPUT_SPECS = {
    "xt": ([128, (IN // 128) * T], BF16),
    "w1t": ([128, (IN // 128) * H], BF16),
    "wi2t": ([128, NCHUNK * H], BF16),
    "wh1": ([128, NCHUNK * 4 * 512], BF16),
    "wh2": ([128, NCHUNK * 4 * 512], BF16),
    "wo2t": ([128, NCHUNK * OUT], BF16),
    "b1": ([1, H], BF16),
    "b2": ([1, H], BF16),
    "bo": ([1, OUT], BF16),
    "ident": ([128, 128], BF16),
    "ones_row": ([1, 128], BF16),
}


def _build(ctx, tc, out_ap, ins):
    nc = tc.nc
    TCH = T // 128

    sb = lambda name, shape, dt: ctx.enter_context(nc.sbuf_tensor(name, shape, dt))

    ident = sb("identsb", [128, 128], BF16)
    nc.sync.dma_start(ident[:], ins["ident"])
    ones_row = sb("onessb", [1, 128], BF16)
    nc.sync.dma_start(ones_row[:], ins["ones_row"])

    a1 = sb("a1sb", [128, TCH * H], BF16)
    a2 = a1  # phases don't overlap: layer-2 A reuses the same buffer
    h1 = sb("h1sb", [128, (T + 1) * 16], BF16)
    h2 = h1  # layer-2 h reuses the same buffer (layer-1 h consumed in phase C)
    nc.vector.memset(h1[:, 0:16], 0.0)

    ts_sb = sb("tssb", [128, 512], F32)  # transpose scratch

    ppool = ctx.enter_context(tc.tile_pool(name="ppool", bufs=2, space="PSUM"))
    bpool = ctx.enter_context(tc.tile_pool(name="bpool", bufs=4, space="PSUM"))

    def batched_proj(a_dst, lhs_of, kchunks, w_sb, bias_sb, tag):
        for tch in range(TCH):
            for ns in range(4):
                pst = bpool.tile([128, 512], F32, tag="pb", name=f"pb_{tag}_{tch}_{ns}")
                ps = pst[0:128, :]
                for kc in range(kchunks):
                    nc.tensor.matmul(
                        ps, lhs_of(kc, tch),
                        w_sb[:, kc * H + ns * 512: kc * H + (ns + 1) * 512],
                        start=(kc == 0), stop=False)
                nc.tensor.matmul(
                    ps, ones_row[:, 0:128], bias_sb[:, ns * 512:(ns + 1) * 512],
                    start=False, stop=True)
                nc.vector.tensor_copy(
                    a_dst[:, tch * H + ns * 512: tch * H + (ns + 1) * 512], ps)

    def recurrence(h_buf, wh_sb, a_sb, tag):
        for t in range(T):
            ps = ppool.tile([128, 512], F32, tag="pz", name=f"pz_{tag}_{t}")
            if t < 2:
                nc.vector.memset(ps[:], 0.0)
            for c in range(17):
                for g in range(4):
                    if c == 0:
                        lhsT = ident[:, t % 128: t % 128 + 1]
                        rhs = a_sb[:, (t // 128) * H + g * 512: (t // 128) * H + (g + 1) * 512]
                    else:
                        cc = c - 1
                        lhsT = h_buf[:, t * 16 + cc: t * 16 + cc + 1]
                        rhs = wh_sb[:, (cc * 4 + g) * 512: (cc * 4 + g + 1) * 512]
                    nc.tensor.matmul(ps[32 * g: 32 * g + 1, :], lhsT, rhs,
                                     start=(c == 0), stop=(c == 16),
                                     tile_position=(0, 32 * g))
            nc.vector.transpose(ts_sb[:], ps[:])
            strided = ts_sb[:].rearrange("p (a b) -> p a b", b=32)[:, :, 0:1]
            nc.scalar.activation(
                h_buf[:, (t + 1) * 16: (t + 2) * 16].unsqueeze(-1), strided, Tanh)

    # ---- phase A ----
    xt_sb = sb("xtsb", [128, (IN // 128) * T], BF16)
    nc.sync.dma_start(xt_sb[:], ins["xt"])
    b1_sb = sb("b1sb", [1, H], BF16)
    nc.sync.dma_start(b1_sb[:], ins["b1"])
    b2_sb = sb("b2sb", [1, H], BF16)
    nc.sync.dma_start(b2_sb[:], ins["b2"])
    bo_sb = sb("bosb", [1, OUT], BF16)
    nc.sync.dma_start(bo_sb[:], ins["bo"])

    wpool = ctx.enter_context(tc.tile_pool(name="wpool", bufs=2))

    w1t_sb = wpool.tile([128, NCHUNK * 4 * 512], BF16, tag="w", name="w1t_t")
    nc.sync.dma_start(w1t_sb[:, 0:(IN // 128) * H], ins["w1t"])
    batched_proj(
        a1, lambda kc, tch: xt_sb[:, kc * T + tch * 128: kc * T + tch * 128 + 128],
        IN // 128, w1t_sb, b1_sb, "a1")

    # ---- phase B ----
    wh1_sb = wpool.tile([128, NCHUNK * 4 * 512], BF16, tag="w", name="wh1_t")
    nc.sync.dma_start(wh1_sb[:], ins["wh1"])
    recurrence(h1, wh1_sb, a1, "l1")

    # ---- phase C ----
    wi2t_sb = wpool.tile([128, NCHUNK * 4 * 512], BF16, tag="w", name="wi2_t")
    nc.sync.dma_start(wi2t_sb[:], ins["wi2t"])
    h1v = h1[:].rearrange("p (t c) -> p t c", c=16)
    batched_proj(
        a2, lambda kc, tch: h1v[:, 1 + tch * 128: 1 + tch * 128 + 128, kc: kc + 1],
        NCHUNK, wi2t_sb, b2_sb, "a2")

    # ---- phase D ----
    wh2_sb = wpool.tile([128, NCHUNK * 4 * 512], BF16, tag="w", name="wh2_t")
    nc.sync.dma_start(wh2_sb[:], ins["wh2"])
    recurrence(h2, wh2_sb, a2, "l2")

    # ---- phase E ----
    wo2t_sb = wpool.tile([128, NCHUNK * 4 * 512], BF16, tag="w", name="wo2_t")
    nc.sync.dma_start(wo2t_sb[:, 0:NCHUNK * OUT], ins["wo2t"])
    out_sb = sb("outsb", [1, OUT], F32)
    for ns in range(2):
        pso = bpool.tile([128, 512], F32, tag="pb", name=f"pso{ns}")
        ps = pso[0:1, :]
        for c in range(NCHUNK):
            nc.tensor.matmul(
                ps, h2[:, T * 16 + c: T * 16 + c + 1],
                wo2t_sb[:, c * OUT + ns * 512: c * OUT + (ns + 1) * 512],
                start=(c == 0), stop=False)
        nc.tensor.matmul(ps, ones_row[:, 0:1], bo_sb[:, ns * 512:(ns + 1) * 512],
                         start=False, stop=True)
        nc.vector.tensor_copy(out_sb[:, ns * 512:(ns + 1) * 512], ps)
    nc.sync.dma_start(out_ap, out_sb[:])


_CACHE = {}


def _get_compiled():
    if "nc" in _CACHE:
        return _CACHE["nc"], _CACHE["in_names"]
    nc = bacc.Bacc("TRN2", target_bir_lowering=False, debug=False, num_devices=8)
    ins = {k: nc.dram_tensor(k, shp, dt, kind="ExternalInput")
           for k, (shp, dt) in _INPUT_SPECS.items()}
    out_dram = nc.dram_tensor("out", [1, OUT], F32, kind="ExternalOutput")
    with tile.TileContext(nc) as tc:
        with ExitStack() as ctx:
            _build(ctx, tc, out_dram.ap(), {k: v.ap() for k, v in ins.items()})
    nc.compile()
    _CACHE["nc"] = nc
    _CACHE["in_names"] = list(ins)
    return nc, list(ins)


def kernel(**inputs) -> np.ndarray:
    prep = _host_prep(inputs)
    nc, in_names = _get_compiled()
    in_map = {k: prep[k] for k in in_names}
    res = bass_utils.run_bass_kernel_spmd(
        nc, [in_map] * 8, core_ids=list(range(8)))
    return np.asarray(res.results[0]["out"], dtype=np.float32)



# revision 4
# speedup vs baseline: 1.1983x; 1.1983x over previous
"""Trainium2 Bass kernel for the 2-layer tanh RNN (nn_DeeperRNN).

Strategy
--------
The T=512 recurrence is inherently serial (batch=1), so the program is
replicated on all 8 NeuronCores (identical SPMD program + data; result read
from core 0).  Structure:

  phase A:  A1 = X @ W_i2h1.T + b1          (batched matmul, upfront)
  block loop (32-step blocks k = 0..15):
     - 32 interleaved step pairs: layer-1 step t = 32k+i back-to-back with
       layer-2 step t' = 32(k-1)+i.  Interleaving the two independent
       recurrence chains keeps the PE busy through each chain's
       transpose/tanh tail (the baseline exposed ~1.4us/step there).
     - C(k): A2 for block k from h1 of block k (W_i2h2 streamed from HBM
       in 16 chunk tiles through a 4-deep pool; ~8MB per block, ~10% of
       HBM bandwidth, fully overlapped).
  final D block + phase E.

Per-step gemv: identical to the baseline (stream the bf16 recurrent weight
matrix through the PE as the moving operand, 4 column-group tiles, DVE
32x32 block transpose, ScalarE tanh, biases/A-terms folded into PSUM via
one-hot / ones rank-1 matmuls).  h histories live in mod-96 rings (96 ≡ 0
mod 32 so 32-step windows never wrap).
"""

import sys
import numpy as np
import ml_dtypes

sys.path.insert(0, "/opt/trn_rl_repo")

import concourse.bass as bass  # noqa: E402
import concourse.mybir as mybir  # noqa: E402
import concourse.bacc as bacc  # noqa: E402
import concourse.tile as tile  # noqa: E402
import concourse.bass_utils as bass_utils  # noqa: E402
from contextlib import ExitStack  # noqa: E402

BF16 = mybir.dt.bfloat16
F32 = mybir.dt.float32
Tanh = mybir.ActivationFunctionType.Tanh

T, IN, H, OUT = 512, 1024, 2048, 1024
NCHUNK = H // 128  # 16
BLK = 32           # interleave block
NBLK = T // BLK    # 16
HR = 96            # h ring length (multiple of BLK; ranges never wrap)


def _host_prep(inputs):
    bf = ml_dtypes.bfloat16
    f32 = np.float32

    def perm_out_axis(a):
        # permute last axis: col (g, J, a2) = g*512 + 32*J + a2 <- row 128J + 32g + a2
        s = a.shape[:-1]
        return np.ascontiguousarray(
            a.reshape(*s, 16, 4, 32).swapaxes(-3, -2).reshape(*s, 2048)
        )

    def prep_wh(w):  # W [j, i] -> [128p, (c*4+g)*512 + J*32 + a2]
        wt = np.asarray(w, f32).T
        return np.ascontiguousarray(
            wt.reshape(16, 128, 16, 4, 32)
            .transpose(1, 0, 3, 2, 4)
            .reshape(128, 16 * 4 * 512)
            .astype(bf)
        )

    def pm(a, part=128):  # [K, N] -> [128, (K//128)*N] chunked partition-major
        k, n = a.shape
        return np.ascontiguousarray(
            a.reshape(k // part, part, n).transpose(1, 0, 2).reshape(part, -1)
        )

    x = np.asarray(inputs["word"], f32).reshape(T, IN)
    return {
        "xt": pm(np.ascontiguousarray(x.T).astype(bf)),
        "w1t": pm(perm_out_axis(np.asarray(inputs["W_i2h1"], f32).T).astype(bf)),
        "wi2t": pm(perm_out_axis(np.asarray(inputs["W_i2h2"], f32).T).astype(bf)),
        "wh1": prep_wh(inputs["W_h2h1"]),
        "wh2": prep_wh(inputs["W_h2h2"]),
        "wo2t": pm(np.asarray(inputs["W_h2o2"], f32).T.astype(bf)),
        "b1": perm_out_axis(
            np.asarray(inputs["b_i2h1"], f32) + np.asarray(inputs["b_h2h1"], f32)
        ).reshape(1, H).astype(bf),
        "b2": perm_out_axis(
            np.asarray(inputs["b_i2h2"], f32) + np.asarray(inputs["b_h2h2"], f32)
        ).reshape(1, H).astype(bf),
        "bo": np.asarray(inputs["b_h2o2"], f32).reshape(1, OUT).astype(bf),
        "ident": np.eye(128, dtype=bf),
        "ones_row": np.ones((1, 128), dtype=bf),
    }


_INPUT_SPECS = {
    "xt": ([128, (IN // 128) * T], BF16),
    "w1t": ([128, (IN // 128) * H], BF16),
    "wi2t": ([128, NCHUNK * H], BF16),
    "wh1": ([128, NCHUNK * 4 * 512], BF16),
    "wh2": ([128, NCHUNK * 4 * 512], BF16),
    "wo2t": ([128, NCHUNK * OUT], BF16),
    "b1": ([1, H], BF16),
    "b2": ([1, H], BF16),
    "bo": ([1, OUT], BF16),
    "ident": ([128, 128], BF16),
    "ones_row": ([1, 128], BF16),
}


def _build(ctx, tc, out_ap, ins):
    nc = tc.nc

    sb = lambda name, shape, dt: ctx.enter_context(nc.sbuf_tensor(name, shape, dt))

    ident = sb("identsb", [128, 128], BF16)
    nc.sync.dma_start(ident[:], ins["ident"])
    ones_row = sb("onessb", [1, 128], BF16)
    nc.sync.dma_start(ones_row[:], ins["ones_row"])
    b1_sb = sb("b1sb", [1, H], BF16)
    nc.sync.dma_start(b1_sb[:], ins["b1"])
    b2_sb = sb("b2sb", [1, H], BF16)
    nc.sync.dma_start(b2_sb[:], ins["b2"])
    bo_sb = sb("bosb", [1, OUT], BF16)
    nc.sync.dma_start(bo_sb[:], ins["bo"])
    xt_sb = sb("xtsb", [128, (IN // 128) * T], BF16)
    nc.sync.dma_start(xt_sb[:], ins["xt"])

    a1 = sb("a1sb", [128, (T // 128) * H], BF16)
    a2 = sb("a2sb", [128, 2 * H], BF16)     # ring of 2 x 128-t tiles
    h1 = sb("h1sb", [128, HR * 16], BF16)   # mod-HR ring
    h2 = sb("h2sb", [128, HR * 16], BF16)
    ts_sb = sb("tssb", [128, 512], F32)
    ts2_sb = sb("ts2sb", [128, 512], F32)
    nc.vector.memset(h1[:, (HR - 1) * 16: HR * 16], 0.0)
    nc.vector.memset(h2[:, (HR - 1) * 16: HR * 16], 0.0)
    nc.vector.memset(a2[:], 0.0)

    wpool = ctx.enter_context(tc.tile_pool(name="wpool", bufs=2))
    ppB = ctx.enter_context(tc.tile_pool(name="ppB", bufs=2, space="PSUM"))
    ppD = ctx.enter_context(tc.tile_pool(name="ppD", bufs=2, space="PSUM"))
    cpool = ctx.enter_context(tc.tile_pool(name="cpool", bufs=1, space="PSUM"))
    wi2pool = ctx.enter_context(tc.tile_pool(name="wi2p", bufs=4))

    def batched_proj(a_dst, lhs_of, kchunks, w_sb, bias_sb, tag):
        for tch in range(T // 128):
            for ns in range(4):
                pst = ppB.tile([128, 512], F32, tag="pzB", name=f"pb_{tag}_{tch}_{ns}")
                ps = pst[0:128, :]
                for kc in range(kchunks):
                    nc.tensor.matmul(
                        ps, lhs_of(kc, tch),
                        w_sb[:, kc * H + ns * 512: kc * H + (ns + 1) * 512],
                        start=(kc == 0), stop=False)
                nc.tensor.matmul(
                    ps, ones_row[:, 0:128], bias_sb[:, ns * 512:(ns + 1) * 512],
                    start=False, stop=True)
                nc.vector.tensor_copy(
                    a_dst[:, tch * H + ns * 512: tch * H + (ns + 1) * 512], ps)

    # ---- phase A ----
    w1t_sb = wpool.tile([128, NCHUNK * 4 * 512], BF16, tag="w", name="w1t_t")
    nc.sync.dma_start(w1t_sb[:, 0:(IN // 128) * H], ins["w1t"])
    batched_proj(
        a1, lambda kc, tch: xt_sb[:, kc * T + tch * 128: kc * T + tch * 128 + 128],
        IN // 128, w1t_sb, b1_sb, "a1")

    wh1_sb = wpool.tile([128, NCHUNK * 4 * 512], BF16, tag="w", name="wh1_t")
    nc.sync.dma_start(wh1_sb[:], ins["wh1"])
    wh2_sb = wpool.tile([128, NCHUNK * 4 * 512], BF16, tag="w", name="wh2_t")
    nc.sync.dma_start(wh2_sb[:], ins["wh2"])

    def chain_step(h_buf, wh_sb, a_sb, a_of, t, pool, tsb, tag):
        pst = pool.tile([128, 512], F32, tag=f"pz{tag}", name=f"pz_{tag}_{t}")
        ps = pst[0:128, :]
        hs = ((t - 1) % HR) * 16
        for c in range(17):
            for g in range(4):
                if c == 0:
                    lhsT = ident[:, t % 128: t % 128 + 1]
                    rhs = a_sb[:, a_of + g * 512: a_of + (g + 1) * 512]
                else:
                    cc = c - 1
                    lhsT = h_buf[:, hs + cc: hs + cc + 1]
                    rhs = wh_sb[:, (cc * 4 + g) * 512: (cc * 4 + g + 1) * 512]
                nc.tensor.matmul(ps[32 * g: 32 * g + 1, :], lhsT, rhs,
                                 start=(c == 0), stop=(c == 16),
                                 tile_position=(0, 32 * g))
        nc.vector.transpose(tsb[:], ps[:])
        strided = tsb[:].rearrange("p (a b) -> p a b", b=32)[:, :, 0:1]
        ho = (t % HR) * 16
        nc.scalar.activation(h_buf[:, ho: ho + 16].unsqueeze(-1), strided, Tanh)

    h1v = h1[:].rearrange("p (t c) -> p t c", c=16)

    def c_block(k):
        # A2 for t in [32k, 32k+32): stationary = h1[32k+1 .. 32k+32]
        r0 = (BLK * k) % 128
        tile_sel = (k // 4) % 2
        pst = cpool.tile([128, 2048], F32, tag="pc", name=f"pc_{k}")
        s0 = (BLK * k) % HR
        for kc in range(NCHUNK):
            wtile = wi2pool.tile([128, H], BF16, tag="wi2", name=f"wi2_{k}_{kc}")
            nc.sync.dma_start(wtile[:], ins["wi2t"][:, kc * H:(kc + 1) * H])
            lhsT = h1v[:, s0: s0 + BLK, kc: kc + 1]
            for g in range(4):
                nc.tensor.matmul(
                    pst[r0: r0 + BLK, g * 512:(g + 1) * 512], lhsT,
                    wtile[:, g * 512:(g + 1) * 512],
                    start=(kc == 0), stop=False, tile_position=(0, r0))
        for g in range(4):
            nc.tensor.matmul(
                pst[r0: r0 + BLK, g * 512:(g + 1) * 512], ones_row[:, 0:BLK],
                b2_sb[:, g * 512:(g + 1) * 512],
                start=False, stop=True, tile_position=(0, r0))
        nc.vector.tensor_copy(
            a2[r0: r0 + BLK, tile_sel * H:(tile_sel + 1) * H],
            pst[r0: r0 + BLK, :])

    # ---- interleaved B/D block loop ----
    for k in range(NBLK + 1):
        for i in range(BLK):
            if k < NBLK:
                t = BLK * k + i
                chain_step(h1, wh1_sb, a1, (t // 128) * H + 0, t, ppB, ts_sb, "B")
            if k > 0:
                tp = BLK * (k - 1) + i
                a_of = ((tp // 128) % 2) * H
                chain_step(h2, wh2_sb, a2, a_of, tp, ppD, ts2_sb, "D")
        if k < NBLK:
            c_block(k)

    # ---- phase E ----
    wo2t_sb = wpool.tile([128, NCHUNK * 4 * 512], BF16, tag="w", name="wo2_t")
    nc.sync.dma_start(wo2t_sb[:, 0:NCHUNK * OUT], ins["wo2t"])
    out_sb = sb("outsb", [1, OUT], F32)
    hfin = ((T - 1) % HR) * 16
    for ns in range(2):
        pso = ppB.tile([128, 512], F32, tag="pzB", name=f"pso{ns}")
        ps = pso[0:1, :]
        for c in range(NCHUNK):
            nc.tensor.matmul(
                ps, h2[:, hfin + c: hfin + c + 1],
                wo2t_sb[:, c * OUT + ns * 512: c * OUT + (ns + 1) * 512],
                start=(c == 0), stop=False)
        nc.tensor.matmul(ps, ones_row[:, 0:1], bo_sb[:, ns * 512:(ns + 1) * 512],
                         start=False, stop=True)
        nc.vector.tensor_copy(out_sb[:, ns * 512:(ns + 1) * 512], ps)
    nc.sync.dma_start(out_ap, out_sb[:])


def _fix_a_of():
    pass


_CACHE = {}


def _get_compiled():
    if "nc" in _CACHE:
        return _CACHE["nc"], _CACHE["in_names"]
    nc = bacc.Bacc("TRN2", target_bir_lowering=False, debug=False, num_devices=8)
    ins = {k: nc.dram_tensor(k, shp, dt, kind="ExternalInput")
           for k, (shp, dt) in _INPUT_SPECS.items()}
    out_dram = nc.dram_tensor("out", [1, OUT], F32, kind="ExternalOutput")
    with tile.TileContext(nc) as tc:
        with ExitStack() as ctx:
            _build(ctx, tc, out_dram.ap(), {k: v.ap() for k, v in ins.items()})
    nc.compile()
    _CACHE["nc"] = nc
    _CACHE["in_names"] = list(ins)
    return nc, list(ins)


def kernel(**inputs) -> np.ndarray:
    prep = _host_prep(inputs)
    nc, in_names = _get_compiled()
    in_map = {k: prep[k] for k in in_names}
    res = bass_utils.run_bass_kernel_spmd(
        nc, [in_map] * 8, core_ids=list(range(8)))
    return np.asarray(res.results[0]["out"], dtype=np.float32)


# revision 8
# speedup vs baseline: 1.2547x; 1.0471x over previous
"""Trainium2 Bass kernel for the 2-layer tanh RNN (nn_DeeperRNN).

Strategy
--------
The T=512 recurrence is inherently serial (batch=1), so the program is
replicated on all 8 NeuronCores (identical SPMD program + data; result read
from core 0).  Structure:

  phase A:  A1 = X @ W_i2h1.T + b1          (batched matmul, upfront)
  block loop (32-step blocks k = 0..15):
     - 32 interleaved step pairs: layer-1 step t = 32k+i back-to-back with
       layer-2 step t' = 32(k-1)+i.  Interleaving the two independent
       recurrence chains keeps the PE busy through each chain's
       transpose/tanh tail (the baseline exposed ~1.4us/step there).
     - C(k): A2 for block k from h1 of block k (W_i2h2 streamed from HBM
       in 16 chunk tiles through a 4-deep pool; ~8MB per block, ~10% of
       HBM bandwidth, fully overlapped).
  final D block + phase E.

Per-step gemv: identical to the baseline (stream the bf16 recurrent weight
matrix through the PE as the moving operand, 4 column-group tiles, DVE
32x32 block transpose, ScalarE tanh, biases/A-terms folded into PSUM via
one-hot / ones rank-1 matmuls).  h histories live in mod-96 rings (96 ≡ 0
mod 32 so 32-step windows never wrap).
"""

import sys
import numpy as np
import ml_dtypes

sys.path.insert(0, "/opt/trn_rl_repo")

import concourse.bass as bass  # noqa: E402
import concourse.mybir as mybir  # noqa: E402
import concourse.bacc as bacc  # noqa: E402
import concourse.tile as tile  # noqa: E402
import concourse.bass_utils as bass_utils  # noqa: E402
from contextlib import ExitStack  # noqa: E402

BF16 = mybir.dt.bfloat16
F32 = mybir.dt.float32
Tanh = mybir.ActivationFunctionType.Tanh

T, IN, H, OUT = 512, 1024, 2048, 1024
NCHUNK = H // 128  # 16
BLK = 32           # interleave block
NBLK = T // BLK    # 16
HR = 96            # h ring length (multiple of BLK; ranges never wrap)


def _host_prep(inputs):
    bf = ml_dtypes.bfloat16
    f32 = np.float32

    def perm_out_axis(a):
        # permute last axis: col (g, J, a2) = g*512 + 32*J + a2 <- row 128J + 32g + a2
        s = a.shape[:-1]
        return np.ascontiguousarray(
            a.reshape(*s, 16, 4, 32).swapaxes(-3, -2).reshape(*s, 2048)
        )

    def prep_wh(w):  # W [j, i] -> [128p, (c*4+g)*512 + J*32 + a2]
        wt = np.asarray(w, f32).T
        return np.ascontiguousarray(
            wt.reshape(16, 128, 16, 4, 32)
            .transpose(1, 0, 3, 2, 4)
            .reshape(128, 16 * 4 * 512)
            .astype(bf)
        )

    def pm(a, part=128):  # [K, N] -> [128, (K//128)*N] chunked partition-major
        k, n = a.shape
        return np.ascontiguousarray(
            a.reshape(k // part, part, n).transpose(1, 0, 2).reshape(part, -1)
        )

    x = np.asarray(inputs["word"], f32).reshape(T, IN)
    return {
        "xt": pm(np.ascontiguousarray(x.T).astype(bf)),
        "w1t": pm(perm_out_axis(np.asarray(inputs["W_i2h1"], f32).T).astype(bf)),
        "wi2t": pm(perm_out_axis(np.asarray(inputs["W_i2h2"], f32).T).astype(bf)),
        "wh1": prep_wh(inputs["W_h2h1"]),
        "wh2": prep_wh(inputs["W_h2h2"]),
        "wo2t": pm(np.asarray(inputs["W_h2o2"], f32).T.astype(bf)),
        "b1": perm_out_axis(
            np.asarray(inputs["b_i2h1"], f32) + np.asarray(inputs["b_h2h1"], f32)
        ).reshape(1, H).astype(bf),
        "b2": perm_out_axis(
            np.asarray(inputs["b_i2h2"], f32) + np.asarray(inputs["b_h2h2"], f32)
        ).reshape(1, H).astype(bf),
        "bo": np.asarray(inputs["b_h2o2"], f32).reshape(1, OUT).astype(bf),
        "ident": np.eye(128, dtype=bf),
        "ones_row": np.ones((1, 128), dtype=bf),
    }


_INPUT_SPECS = {
    "xt": ([128, (IN // 128) * T], BF16),
    "w1t": ([128, (IN // 128) * H], BF16),
    "wi2t": ([128, NCHUNK * H], BF16),
    "wh1": ([128, NCHUNK * 4 * 512], BF16),
    "wh2": ([128, NCHUNK * 4 * 512], BF16),
    "wo2t": ([128, NCHUNK * OUT], BF16),
    "b1": ([1, H], BF16),
    "b2": ([1, H], BF16),
    "bo": ([1, OUT], BF16),
    "ident": ([128, 128], BF16),
    "ones_row": ([1, 128], BF16),
}


def _build(ctx, tc, out_ap, ins):
    nc = tc.nc

    sb = lambda name, shape, dt: ctx.enter_context(nc.sbuf_tensor(name, shape, dt))

    ident = sb("identsb", [128, 128], BF16)
    nc.sync.dma_start(ident[:], ins["ident"])
    ones_row = sb("onessb", [1, 128], BF16)
    nc.sync.dma_start(ones_row[:], ins["ones_row"])
    b1_sb = sb("b1sb", [1, H], BF16)
    nc.sync.dma_start(b1_sb[:], ins["b1"])
    b2_sb = sb("b2sb", [1, H], BF16)
    nc.sync.dma_start(b2_sb[:], ins["b2"])
    bo_sb = sb("bosb", [1, OUT], BF16)
    nc.sync.dma_start(bo_sb[:], ins["bo"])
    xt_sb = sb("xtsb", [128, (IN // 128) * T], BF16)
    nc.sync.dma_start(xt_sb[:], ins["xt"])

    a1 = sb("a1sb", [128, (T // 128) * H], BF16)
    a2 = sb("a2sb", [128, 2 * H], BF16)     # ring of 2 x 128-t tiles
    h1 = sb("h1sb", [128, HR * 16], BF16)   # mod-HR ring
    h2 = sb("h2sb", [128, HR * 16], BF16)
    ts_sb = sb("tssb", [128, 512], F32)
    ts2_sb = sb("ts2sb", [128, 512], F32)
    nc.vector.memset(h1[:, (HR - 1) * 16: HR * 16], 0.0)
    nc.vector.memset(h2[:, (HR - 1) * 16: HR * 16], 0.0)
    nc.vector.memset(a2[:], 0.0)

    wpool = ctx.enter_context(tc.tile_pool(name="wpool", bufs=2))
    ppB = ctx.enter_context(tc.tile_pool(name="ppB", bufs=2, space="PSUM"))
    ppD = ctx.enter_context(tc.tile_pool(name="ppD", bufs=2, space="PSUM"))
    cpool = ctx.enter_context(tc.tile_pool(name="cpool", bufs=1, space="PSUM"))
    wi2pool = ctx.enter_context(tc.tile_pool(name="wi2p", bufs=4))

    def batched_proj(a_dst, lhs_of, kchunks, w_sb, bias_sb, tag):
        for tch in range(T // 128):
            for ns in range(4):
                pst = ppB.tile([128, 512], F32, tag="pzB", name=f"pb_{tag}_{tch}_{ns}")
                ps = pst[0:128, :]
                for kc in range(kchunks):
                    nc.tensor.matmul(
                        ps, lhs_of(kc, tch),
                        w_sb[:, kc * H + ns * 512: kc * H + (ns + 1) * 512],
                        start=(kc == 0), stop=False)
                nc.tensor.matmul(
                    ps, ones_row[:, 0:128], bias_sb[:, ns * 512:(ns + 1) * 512],
                    start=False, stop=True)
                nc.vector.tensor_copy(
                    a_dst[:, tch * H + ns * 512: tch * H + (ns + 1) * 512], ps)

    # ---- phase A ----
    w1t_sb = wpool.tile([128, NCHUNK * 4 * 512], BF16, tag="w", name="w1t_t")
    nc.sync.dma_start(w1t_sb[:, 0:(IN // 128) * H], ins["w1t"])
    batched_proj(
        a1, lambda kc, tch: xt_sb[:, kc * T + tch * 128: kc * T + tch * 128 + 128],
        IN // 128, w1t_sb, b1_sb, "a1")

    wh1_sb = wpool.tile([128, NCHUNK * 4 * 512], BF16, tag="w", name="wh1_t")
    nc.sync.dma_start(wh1_sb[:], ins["wh1"])
    wh2_sb = wpool.tile([128, NCHUNK * 4 * 512], BF16, tag="w", name="wh2_t")
    nc.sync.dma_start(wh2_sb[:], ins["wh2"])

    def chain_step(h_buf, wh_sb, a_sb, a_of, t, pool, tsb, tag, a2x=False):
        pst = pool.tile([128, 512], F32, tag=f"pz{tag}", name=f"pz_{tag}_{t}")
        ps = pst[0:128, :]
        hs = ((t - 1) % HR) * 16
        for c in range(17):
            for g in range(4):
                if c == 0:
                    icol = 32 * g + (t % BLK) if a2x else t % 128
                    lhsT = ident[:, icol: icol + 1]
                    rhs = a_sb[:, a_of + g * 512: a_of + (g + 1) * 512]
                else:
                    cc = c - 1
                    lhsT = h_buf[:, hs + cc: hs + cc + 1]
                    rhs = wh_sb[:, (cc * 4 + g) * 512: (cc * 4 + g + 1) * 512]
                nc.tensor.matmul(ps[32 * g: 32 * g + 1, :], lhsT, rhs,
                                 start=(c == 0), stop=(c == 16),
                                 tile_position=(0, 32 * g))
        nc.vector.transpose(tsb[:], ps[:])
        strided = tsb[:].rearrange("p (a b) -> p a b", b=32)[:, :, 0:1]
        ho = (t % HR) * 16
        nc.scalar.activation(h_buf[:, ho: ho + 16].unsqueeze(-1), strided, Tanh)

    h1v = h1[:].rearrange("p (t c) -> p t c", c=16)
    cps = {}

    def c_chunk(kblk, kc):
        # A2 for t in [32*kblk, 32*kblk+32): 4 output groups concurrent,
        # group g at PE col-group g -> psum rows [32g, 32g+32).
        if kc == 0:
            cps[kblk] = cpool.tile([128, 2048], F32, tag="pc", name=f"pc_{kblk}")
        pst = cps[kblk]
        s0 = (BLK * kblk) % HR
        wtile = wi2pool.tile([128, H], BF16, tag="wi2", name=f"wi2_{kblk}_{kc}")
        nc.sync.dma_start(wtile[:], ins["wi2t"][:, kc * H:(kc + 1) * H])
        lhsT = h1v[:, s0: s0 + BLK, kc: kc + 1]
        for g in range(4):
            nc.tensor.matmul(
                pst[32 * g: 32 * g + BLK, g * 512:(g + 1) * 512], lhsT,
                wtile[:, g * 512:(g + 1) * 512],
                start=(kc == 0), stop=False, tile_position=(0, 32 * g))

    def c_finish(kblk):
        pst = cps.pop(kblk)
        slot = kblk % 2
        for g in range(4):
            nc.tensor.matmul(
                pst[32 * g: 32 * g + BLK, g * 512:(g + 1) * 512],
                ones_row[:, 0:BLK], b2_sb[:, g * 512:(g + 1) * 512],
                start=False, stop=True, tile_position=(0, 32 * g))
        for g in range(4):
            nc.vector.tensor_copy(
                a2[32 * g: 32 * g + BLK, slot * H + g * 512: slot * H + (g + 1) * 512],
                pst[32 * g: 32 * g + BLK, g * 512:(g + 1) * 512])

    # ---- interleaved B/D block loop (D lags 2 blocks; C woven in) ----
    LAG = 2
    for k in range(NBLK + LAG):
        for i in range(BLK):
            if k < NBLK:
                t = BLK * k + i
                chain_step(h1, wh1_sb, a1, (t // 128) * H, t, ppB, ts_sb, "B")
            if 1 <= k <= NBLK and i % 2 == 0:
                c_chunk(k - 1, i // 2)
            if k >= LAG:
                tp = BLK * (k - LAG) + i
                a_of = ((tp // BLK) % 2) * H
                chain_step(h2, wh2_sb, a2, a_of, tp, ppD, ts2_sb, "D", a2x=True)
        if 1 <= k <= NBLK:
            c_finish(k - 1)

    # ---- phase E ----
    wo2t_sb = wpool.tile([128, NCHUNK * 4 * 512], BF16, tag="w", name="wo2_t")
    nc.sync.dma_start(wo2t_sb[:, 0:NCHUNK * OUT], ins["wo2t"])
    out_sb = sb("outsb", [1, OUT], F32)
    hfin = ((T - 1) % HR) * 16
    for ns in range(2):
        pso = ppB.tile([128, 512], F32, tag="pzB", name=f"pso{ns}")
        ps = pso[0:1, :]
        for c in range(NCHUNK):
            nc.tensor.matmul(
                ps, h2[:, hfin + c: hfin + c + 1],
                wo2t_sb[:, c * OUT + ns * 512: c * OUT + (ns + 1) * 512],
                start=(c == 0), stop=False)
        nc.tensor.matmul(ps, ones_row[:, 0:1], bo_sb[:, ns * 512:(ns + 1) * 512],
                         start=False, stop=True)
        nc.vector.tensor_copy(out_sb[:, ns * 512:(ns + 1) * 512], ps)
    nc.sync.dma_start(out_ap, out_sb[:])


def _fix_a_of():
    pass


_CACHE = {}


def _get_compiled():
    if "nc" in _CACHE:
        return _CACHE["nc"], _CACHE["in_names"]
    nc = bacc.Bacc("TRN2", target_bir_lowering=False, debug=False, num_devices=8)
    ins = {k: nc.dram_tensor(k, shp, dt, kind="ExternalInput")
           for k, (shp, dt) in _INPUT_SPECS.items()}
    out_dram = nc.dram_tensor("out", [1, OUT], F32, kind="ExternalOutput")
    with tile.TileContext(nc) as tc:
        with ExitStack() as ctx:
            _build(ctx, tc, out_dram.ap(), {k: v.ap() for k, v in ins.items()})
    nc.compile()
    _CACHE["nc"] = nc
    _CACHE["in_names"] = list(ins)
    return nc, list(ins)


def kernel(**inputs) -> np.ndarray:
    prep = _host_prep(inputs)
    nc, in_names = _get_compiled()
    in_map = {k: prep[k] for k in in_names}
    res = bass_utils.run_bass_kernel_spmd(
        nc, [in_map] * 8, core_ids=list(range(8)))
    return np.asarray(res.results[0]["out"], dtype=np.float32)


# revision 12
# speedup vs baseline: 1.3038x; 1.0391x over previous
"""Trainium2 Bass kernel for the 2-layer tanh RNN (nn_DeeperRNN).

Strategy
--------
The T=512 recurrence is inherently serial (batch=1), so the program is
replicated on all 8 NeuronCores (identical SPMD program + data; result read
from core 0).  Structure:

  phase A:  A1 = X @ W_i2h1.T + b1          (batched matmul, upfront)
  block loop (32-step blocks k = 0..15):
     - 32 interleaved step pairs: layer-1 step t = 32k+i back-to-back with
       layer-2 step t' = 32(k-1)+i.  Interleaving the two independent
       recurrence chains keeps the PE busy through each chain's
       transpose/tanh tail (the baseline exposed ~1.4us/step there).
     - C(k): A2 for block k from h1 of block k (W_i2h2 streamed from HBM
       in 16 chunk tiles through a 4-deep pool; ~8MB per block, ~10% of
       HBM bandwidth, fully overlapped).
  final D block + phase E.

Per-step gemv: identical to the baseline (stream the bf16 recurrent weight
matrix through the PE as the moving operand, 4 column-group tiles, DVE
32x32 block transpose, ScalarE tanh, biases/A-terms folded into PSUM via
one-hot / ones rank-1 matmuls).  h histories live in mod-96 rings (96 ≡ 0
mod 32 so 32-step windows never wrap).
"""

import sys
import numpy as np
import ml_dtypes

sys.path.insert(0, "/opt/trn_rl_repo")

import concourse.bass as bass  # noqa: E402
import concourse.mybir as mybir  # noqa: E402
import concourse.bacc as bacc  # noqa: E402
import concourse.tile as tile  # noqa: E402
import concourse.bass_utils as bass_utils  # noqa: E402
from contextlib import ExitStack  # noqa: E402

BF16 = mybir.dt.bfloat16
F32 = mybir.dt.float32
Tanh = mybir.ActivationFunctionType.Tanh

T, IN, H, OUT = 512, 1024, 2048, 1024
NCHUNK = H // 128  # 16
BLK = 32           # interleave block
NBLK = T // BLK    # 16
HR = 96            # h ring length (multiple of BLK; ranges never wrap)


def _host_prep(inputs):
    bf = ml_dtypes.bfloat16
    f32 = np.float32

    def perm_out_axis(a):
        # permute last axis: col (g, J, a2) = g*512 + 32*J + a2 <- row 128J + 32g + a2
        s = a.shape[:-1]
        return np.ascontiguousarray(
            a.reshape(*s, 16, 4, 32).swapaxes(-3, -2).reshape(*s, 2048)
        )

    def prep_wh(w):  # W [j, i] -> [128p, (c*4+g)*512 + J*32 + a2]
        wt = np.asarray(w, f32).T
        return np.ascontiguousarray(
            wt.reshape(16, 128, 16, 4, 32)
            .transpose(1, 0, 3, 2, 4)
            .reshape(128, 16 * 4 * 512)
            .astype(bf)
        )

    def pm(a, part=128):  # [K, N] -> [128, (K//128)*N] chunked partition-major
        k, n = a.shape
        return np.ascontiguousarray(
            a.reshape(k // part, part, n).transpose(1, 0, 2).reshape(part, -1)
        )

    x = np.asarray(inputs["word"], f32).reshape(T, IN)
    return {
        "xt": pm(np.ascontiguousarray(x.T).astype(bf)),
        "w1t": pm(perm_out_axis(np.asarray(inputs["W_i2h1"], f32).T).astype(bf)),
        "wi2t": pm(perm_out_axis(np.asarray(inputs["W_i2h2"], f32).T).astype(bf)),
        "wh1": prep_wh(inputs["W_h2h1"]),
        "wh2": prep_wh(inputs["W_h2h2"]),
        "wo2t": pm(np.asarray(inputs["W_h2o2"], f32).T.astype(bf)),
        "b1": perm_out_axis(
            np.asarray(inputs["b_i2h1"], f32) + np.asarray(inputs["b_h2h1"], f32)
        ).reshape(1, H).astype(bf),
        "b2": perm_out_axis(
            np.asarray(inputs["b_i2h2"], f32) + np.asarray(inputs["b_h2h2"], f32)
        ).reshape(1, H).astype(bf),
        "bo": np.asarray(inputs["b_h2o2"], f32).reshape(1, OUT).astype(bf),
        "ident": np.eye(128, dtype=bf),
        "ones_row": np.ones((1, 128), dtype=bf),
    }


_INPUT_SPECS = {
    "xt": ([128, (IN // 128) * T], BF16),
    "w1t": ([128, (IN // 128) * H], BF16),
    "wi2t": ([128, NCHUNK * H], BF16),
    "wh1": ([128, NCHUNK * 4 * 512], BF16),
    "wh2": ([128, NCHUNK * 4 * 512], BF16),
    "wo2t": ([128, NCHUNK * OUT], BF16),
    "b1": ([1, H], BF16),
    "b2": ([1, H], BF16),
    "bo": ([1, OUT], BF16),
    "ident": ([128, 128], BF16),
    "ones_row": ([1, 128], BF16),
}


def _build(ctx, tc, out_ap, ins):
    nc = tc.nc

    sb = lambda name, shape, dt: ctx.enter_context(nc.sbuf_tensor(name, shape, dt))

    ident = sb("identsb", [128, 128], BF16)
    nc.sync.dma_start(ident[:], ins["ident"])
    ones_row = sb("onessb", [1, 128], BF16)
    nc.sync.dma_start(ones_row[:], ins["ones_row"])
    b1_sb = sb("b1sb", [1, H], BF16)
    nc.sync.dma_start(b1_sb[:], ins["b1"])
    b2_sb = sb("b2sb", [1, H], BF16)
    nc.sync.dma_start(b2_sb[:], ins["b2"])
    bo_sb = sb("bosb", [1, OUT], BF16)
    nc.sync.dma_start(bo_sb[:], ins["bo"])
    xt_sb = sb("xtsb", [128, (IN // 128) * T], BF16)
    nc.sync.dma_start(xt_sb[:], ins["xt"])

    a1 = sb("a1sb", [128, (T // 128) * H], BF16)
    a2t = sb("a2tsb", [128, 2 * 512], F32)  # A2^T ring: [32g+a2, 16*j+J] per block
    h1 = sb("h1sb", [128, HR * 16], BF16)   # mod-HR ring
    h2 = sb("h2sb", [128, HR * 16], BF16)
    ts_sb = sb("tssb", [128, 512], F32)
    ts2_sb = sb("ts2sb", [128, 512], F32)
    tsc_sb = sb("tscsb", [128, 512], F32)
    nc.vector.memset(h1[:, (HR - 1) * 16: HR * 16], 0.0)
    nc.vector.memset(h2[:, (HR - 1) * 16: HR * 16], 0.0)

    wpool = ctx.enter_context(tc.tile_pool(name="wpool", bufs=2))
    ppB = ctx.enter_context(tc.tile_pool(name="ppB", bufs=3, space="PSUM"))
    ppD = ctx.enter_context(tc.tile_pool(name="ppD", bufs=3, space="PSUM"))
    cpool = ctx.enter_context(tc.tile_pool(name="cpool", bufs=1, space="PSUM"))
    wi2pool = ctx.enter_context(tc.tile_pool(name="wi2p", bufs=4))

    def batched_proj(a_dst, lhs_of, kchunks, w_sb, bias_sb, tag):
        for tch in range(T // 128):
            for ns in range(4):
                pst = ppB.tile([128, 512], F32, tag="pzB", name=f"pb_{tag}_{tch}_{ns}")
                ps = pst[0:128, :]
                for kc in range(kchunks):
                    nc.tensor.matmul(
                        ps, lhs_of(kc, tch),
                        w_sb[:, kc * H + ns * 512: kc * H + (ns + 1) * 512],
                        start=(kc == 0), stop=False)
                nc.tensor.matmul(
                    ps, ones_row[:, 0:128], bias_sb[:, ns * 512:(ns + 1) * 512],
                    start=False, stop=True)
                nc.vector.tensor_copy(
                    a_dst[:, tch * H + ns * 512: tch * H + (ns + 1) * 512], ps)

    # ---- phase A ----
    w1t_sb = wpool.tile([128, NCHUNK * 4 * 512], BF16, tag="w", name="w1t_t")
    nc.sync.dma_start(w1t_sb[:, 0:(IN // 128) * H], ins["w1t"])
    batched_proj(
        a1, lambda kc, tch: xt_sb[:, kc * T + tch * 128: kc * T + tch * 128 + 128],
        IN // 128, w1t_sb, b1_sb, "a1")

    wh1_sb = wpool.tile([128, NCHUNK * 4 * 512], BF16, tag="w", name="wh1_t")
    nc.sync.dma_start(wh1_sb[:], ins["wh1"])
    wh2_sb = wpool.tile([128, NCHUNK * 4 * 512], BF16, tag="w", name="wh2_t")
    nc.sync.dma_start(wh2_sb[:], ins["wh2"])

    def chain_step(h_buf, wh_sb, a_sb, a_of, t, pool, tsb, tag, aT=None):
        pst = pool.tile([128, 512], F32, tag=f"pz{tag}", name=f"pz_{tag}_{t}")
        ps = pst[0:128, :]
        hs = ((t - 1) % HR) * 16
        nslot = 16 if aT is not None else 17
        for c in range(nslot):
            for g in range(4):
                if aT is None and c == 0:
                    lhsT = ident[:, t % 128: t % 128 + 1]
                    rhs = a_sb[:, a_of + g * 512: a_of + (g + 1) * 512]
                else:
                    cc = c - 1 if aT is None else c
                    lhsT = h_buf[:, hs + cc: hs + cc + 1]
                    rhs = wh_sb[:, (cc * 4 + g) * 512: (cc * 4 + g + 1) * 512]
                nc.tensor.matmul(ps[32 * g: 32 * g + 1, :], lhsT, rhs,
                                 start=(c == 0), stop=(c == nslot - 1),
                                 tile_position=(0, 32 * g))
        nc.vector.transpose(tsb[:], ps[:])
        strided = tsb[:].rearrange("p (a b) -> p a b", b=32)[:, :, 0:1]
        ho = (t % HR) * 16
        if aT is None:
            nc.scalar.activation(h_buf[:, ho: ho + 16].unsqueeze(-1), strided, Tanh)
        else:
            tsum = tspool.tile([128, 16], F32, tag="tsum", name=f"tsum_{t}")
            nc.vector.tensor_tensor(
                tsum[:].unsqueeze(-1), strided,
                aT[:, :].unsqueeze(-1), mybir.AluOpType.add)
            nc.scalar.activation(h_buf[:, ho: ho + 16], tsum[:], Tanh)

    h1v = h1[:].rearrange("p (t c) -> p t c", c=16)
    cps = {}

    def c_chunk(kblk, kc):
        # A2 for t in [32*kblk, 32*kblk+32): 4 output groups concurrent,
        # group g at PE col-group g -> psum rows [32g, 32g+32).
        if kc == 0:
            cps[kblk] = cpool.tile([128, 512], F32, tag="pc", name=f"pc_{kblk}")
        pst = cps[kblk]
        s0 = (BLK * kblk) % HR
        wtile = wi2pool.tile([128, H], BF16, tag="wi2", name=f"wi2_{kblk}_{kc}")
        nc.sync.dma_start(wtile[:], ins["wi2t"][:, kc * H:(kc + 1) * H])
        lhsT = h1v[:, s0: s0 + BLK, kc: kc + 1]
        for g in range(4):
            nc.tensor.matmul(
                pst[32 * g: 32 * g + BLK, 0:512], lhsT,
                wtile[:, g * 512:(g + 1) * 512],
                start=(kc == 0), stop=False, tile_position=(0, 32 * g))

    def c_finish(kblk):
        pst = cps.pop(kblk)
        slot = kblk % 2
        for g in range(4):
            nc.tensor.matmul(
                pst[32 * g: 32 * g + BLK, 0:512],
                ones_row[:, 0:BLK], b2_sb[:, g * 512:(g + 1) * 512],
                start=False, stop=True, tile_position=(0, 32 * g))
        nc.vector.transpose(tsc_sb[:], pst[0:128, :])
        nc.vector.tensor_copy(
            a2t[:, slot * 512:(slot + 1) * 512].rearrange("p (j J) -> p j J", J=16),
            tsc_sb[:].rearrange("p (J j) -> p j J", j=32))

    tspool = ctx.enter_context(tc.tile_pool(name="tspool", bufs=4))

    # ---- interleaved B/D block loop (D lags 2 blocks; C woven in) ----
    LAG = 2
    for k in range(NBLK + LAG):
        for i in range(BLK):
            if k < NBLK:
                t = BLK * k + i
                chain_step(h1, wh1_sb, a1, (t // 128) * H, t, ppB, ts_sb, "B")
            if 1 <= k <= NBLK and i % 2 == 0:
                c_chunk(k - 1, i // 2)
            if k >= LAG:
                tp = BLK * (k - LAG) + i
                slot = (tp // BLK) % 2
                aTs = a2t[:, slot * 512 + (tp % BLK) * 16: slot * 512 + (tp % BLK) * 16 + 16]
                chain_step(h2, wh2_sb, None, 0, tp, ppD, ts2_sb, "D", aT=aTs)
        if 1 <= k <= NBLK:
            c_finish(k - 1)

    # ---- phase E ----
    wo2t_sb = wpool.tile([128, NCHUNK * 4 * 512], BF16, tag="w", name="wo2_t")
    nc.sync.dma_start(wo2t_sb[:, 0:NCHUNK * OUT], ins["wo2t"])
    out_sb = sb("outsb", [1, OUT], F32)
    hfin = ((T - 1) % HR) * 16
    for ns in range(2):
        pso = ppB.tile([128, 512], F32, tag="pzB", name=f"pso{ns}")
        ps = pso[0:1, :]
        for c in range(NCHUNK):
            nc.tensor.matmul(
                ps, h2[:, hfin + c: hfin + c + 1],
                wo2t_sb[:, c * OUT + ns * 512: c * OUT + (ns + 1) * 512],
                start=(c == 0), stop=False)
        nc.tensor.matmul(ps, ones_row[:, 0:1], bo_sb[:, ns * 512:(ns + 1) * 512],
                         start=False, stop=True)
        nc.vector.tensor_copy(out_sb[:, ns * 512:(ns + 1) * 512], ps)
    nc.sync.dma_start(out_ap, out_sb[:])


def _fix_a_of():
    pass


_CACHE = {}


def _get_compiled():
    if "nc" in _CACHE:
        return _CACHE["nc"], _CACHE["in_names"]
    nc = bacc.Bacc("TRN2", target_bir_lowering=False, debug=False, num_devices=8)
    ins = {k: nc.dram_tensor(k, shp, dt, kind="ExternalInput")
           for k, (shp, dt) in _INPUT_SPECS.items()}
    out_dram = nc.dram_tensor("out", [1, OUT], F32, kind="ExternalOutput")
    with tile.TileContext(nc) as tc:
        with ExitStack() as ctx:
            _build(ctx, tc, out_dram.ap(), {k: v.ap() for k, v in ins.items()})
    nc.compile()
    _CACHE["nc"] = nc
    _CACHE["in_names"] = list(ins)
    return nc, list(ins)


def kernel(**inputs) -> np.ndarray:
    prep = _host_prep(inputs)
    nc, in_names = _get_compiled()
    in_map = {k: prep[k] for k in in_names}
    res = bass_utils.run_bass_kernel_spmd(
        nc, [in_map] * 8, core_ids=list(range(8)))
    return np.asarray(res.results[0]["out"], dtype=np.float32)
